# revision 24
# baseline (speedup 1.0000x reference)
"""Trainium2 Bass kernel for nn_AFDEKM_Model (attention-gated encoder + fuzzy
c-means membership), data-parallel over 8 NeuronCores.

Layout: feature-major on device — activations live as [features, batch_tile]
so Linear layers need no transposes (weights [in, out] are already the lhsT
the TensorEngine wants), BatchNorm batch-reductions are free-axis DVE ops, and
BN apply is one fused ScalarEngine activation per tile. Softmax over features
(partition axis) uses ones-matmul column sums plus a K=1 broadcast matmul.
The host transposes x once and transposes the three outputs back.

Matmul operands are bf16 (full-rate PE path; measured 216ns/MM warm at N=512),
PSUM accumulation fp32. Tiles are processed in groups with software-pipelined
emission (group g's normalize/encode stage is emitted after group g+1's
matmul stages) so the TensorEngine never idles long enough for the HAM clock
gate to re-throttle it, and the ScalarEngine runs same-LUT activations
back-to-back (table reloads cost ~1.3us). Training-mode BatchNorm over the
full 65536-row batch: per-core partial stats are combined with two tiny
AllReduces.
"""

import sys
import types

import numpy as np

B, D_IN, D_MID, D_HID, K = 65536, 512, 256, 64, 64
BN_EPS = 1e-5
NCORES = 8
BS = B // NCORES          # 8192 rows per core
BT = 512                  # batch tile (free dim; == PSUM bank / BN_STATS_FMAX)
NT = BS // BT             # 16 tiles per core
GRP = 4                   # tiles per ACT-table group
PCH = 128                 # partition chunk

_DIN_CH = D_IN // PCH     # 4
_DMID_CH = D_MID // PCH   # 2


def _shim_antenv_hooks():
    # this container's antenv lacks axon_hooks; bass_utils imports it when
    # trace=True. Harmless no-op module unless test.py installs the real one.
    if "antenv.axon_hooks" not in sys.modules:
        mod = types.ModuleType("antenv.axon_hooks")
        holder = {}
        mod.set_axon_ntff_profile_hook = lambda h: holder.__setitem__("h", h)
        mod.get_axon_ntff_profile_hook = lambda: holder.get("h")
        sys.modules["antenv.axon_hooks"] = mod


_shim_antenv_hooks()

import concourse.mybir as mybir  # noqa: E402
import concourse.bass as bass  # noqa: E402
import concourse.tile as tile  # noqa: E402
from concourse.bass_utils import run_bass_kernel_spmd  # noqa: E402

F32 = mybir.dt.float32
BF16 = mybir.dt.bfloat16
AF = mybir.ActivationFunctionType
ALU = mybir.AluOpType

_MAX_WAITS = 1  # this neuronxcc build allows one sync-wait per instruction
_wait_ctr = [0]


def _split_excess_waits(nc):
    """Spill excess semaphore waits onto same-engine NoOps (walrus here
    rejects instructions with >1 sync wait)."""
    for fn in nc.m.functions:
        for bb in fn.blocks:
            idx = 0
            while idx < len(bb.instructions):
                ins = bb.instructions[idx]
                si = ins.sync_info
                if si is None:
                    idx += 1
                    continue
                waits = list(si.on_wait)
                if len(waits) <= _MAX_WAITS:
                    idx += 1
                    continue
                keep = waits[-_MAX_WAITS:]
                extra = waits[:-_MAX_WAITS]
                n_ins = 0
                for i in range(0, len(extra), _MAX_WAITS):
                    chunk = extra[i : i + _MAX_WAITS]
                    _wait_ctr[0] += 1
                    nop = mybir.InstNoOp(
                        name=f"{ins.name}-wspill{_wait_ctr[0]}",
                        sync_info=mybir.SyncInfo(on_wait=chunk, on_update=[]),
                        bass_nofuse=True,
                        engine=ins.engine,
                    )
                    nc.register_instruction(nop, overwrite=True)
                    bb.instructions.insert(idx + n_ins, nop)
                    n_ins += 1
                ins.sync_info = mybir.SyncInfo(
                    on_wait=keep, on_update=list(si.on_update)
                )
                idx += n_ins + 1


def _recip(nc, out_ap, in_ap):
    """1/x on the ScalarEngine LUT (HW-measured ~1e-5 max rel err over
    1e-2..1e6 — plenty for softmax/membership normalizers). bass's
    activation() refuses Reciprocal wholesale, so emit InstActivation
    directly."""
    eng = nc.scalar
    inputs = [eng.lower_ap(in_ap)]
    for arg in (0.0, 1.0, 0.0):  # bias, scale, alpha
        inputs.append(mybir.ImmediateValue(dtype=mybir.dt.float32, value=float(arg)))
    return eng.add_instruction(
        mybir.InstActivation(
            name=eng.bass.get_next_instruction_name(),
            func=AF.Reciprocal,
            ins=inputs,
            outs=[eng.lower_ap(out_ap)],
        )
    )


def build():
    nc = bass.Bass()

    # ---- I/O ----
    xT = nc.declare_dram_parameter("xT", [D_IN, BS], BF16, isOutput=False)
    # all [128, *] weights packed into one tensor -> one startup DMA
    # cols: aw1 (4x512) | aw2 (4x512) | ew1 (4x256) | ew2 (2x64) | ones_col (1)
    WPACK = 4 * D_IN + 4 * D_IN + 4 * D_MID + 2 * D_HID + 1
    wpack = nc.declare_dram_parameter("wpack", [PCH, WPACK], BF16, isOutput=False)
    # biases/gammas packed: ab1 (4) | ab2 (4) | eb1 (2) | g1 (2) | b1 (2) |
    # eb2 | g2 | b2  (last three on partitions 0:64)
    bpack = nc.declare_dram_parameter("bpack", [PCH, 17], F32, isOutput=False)
    caug = nc.declare_dram_parameter("caug", [D_HID + 2, K], BF16, isOutput=False)
    # ones_row (128) | ones_bt (512) on one partition
    rpack = nc.declare_dram_parameter("rpack", [1, PCH + BT], BF16, isOutput=False)
    # encoder layer-2 weights in float32r: the e2 matmul is tiny (2 per tile)
    # and running it at full fp32 precision halves enc's worst-case error
    F32R = mybir.dt.float32r
    ew2r = nc.declare_dram_parameter("ew2r", [D_MID, D_HID], F32R, isOutput=False)

    attnT = nc.declare_dram_parameter("attnT", [D_IN, BS], BF16, isOutput=True)
    encT = nc.declare_dram_parameter("encT", [D_HID, BS], BF16, isOutput=True)
    membT = nc.declare_dram_parameter("membT", [D_HID, BS], BF16, isOutput=True)

    with tile.TileContext(nc) as tc:
        with (
            tc.tile_pool(name="wp", bufs=1) as wp,
            tc.tile_pool(name="rp", bufs=1) as rp,
            tc.tile_pool(name="tp", bufs=2) as tp,
            tc.tile_pool(name="pmm", bufs=4, space="PSUM") as pmm,
            tc.tile_pool(name="ps1", bufs=4, space="PSUM") as ps1,
            tc.tile_pool(name="prb", bufs=1, space="PSUM") as prb,
            tc.tile_pool(name="dp", bufs=1, space="DRAM") as dp,
        ):
            # ---- load weights / constants (one DMA each) ----
            wpack_sb = wp.tile([PCH, 4 * D_IN + 4 * D_IN + 4 * D_MID + 2 * D_HID + 1], BF16, name="wpack_sb")
            nc.sync.dma_start(out=wpack_sb[:, :], in_=wpack[:, :])
            bpack_sb = wp.tile([PCH, 17], F32, name="bpack_sb")
            nc.sync.dma_start(out=bpack_sb[:, :], in_=bpack[:, :])
            caug_sb = wp.tile([D_HID + 2, K], BF16, name="caug_sb")
            nc.sync.dma_start(out=caug_sb[:, :], in_=caug[:, :])
            rpack_sb = wp.tile([1, PCH + BT], BF16, name="rpack_sb")
            nc.sync.dma_start(out=rpack_sb[:, :], in_=rpack[:, :])
            ew2r_sb = wp.tile([PCH, _DMID_CH, D_HID], F32R, name="ew2r_sb")
            nc.sync.dma_start(
                out=ew2r_sb[:, :, :],
                in_=ew2r[:, :].rearrange("(c p) h -> p c h", c=_DMID_CH),
            )

            o = [0]

            def _wslice(n):
                a = wpack_sb[:, o[0] : o[0] + n]
                o[0] += n
                return a

            aw1_sb = [_wslice(D_IN) for _ in range(_DIN_CH)]
            aw2_sb = [_wslice(D_IN) for _ in range(_DIN_CH)]
            ew1_sb = [_wslice(D_MID) for _ in range(_DIN_CH)]
            ew2_sb = [_wslice(D_HID) for _ in range(_DMID_CH)]
            onec_sb = _wslice(1)
            ab1_sb = [bpack_sb[:, m : m + 1] for m in range(4)]
            ab2_sb = [bpack_sb[:, 4 + m : 5 + m] for m in range(4)]
            eb1_sb = [bpack_sb[:, 8 + m : 9 + m] for m in range(2)]
            g1_sb = [bpack_sb[:, 10 + m : 11 + m] for m in range(2)]
            b1_sb = [bpack_sb[:, 12 + m : 13 + m] for m in range(2)]
            eb2_sb = bpack_sb[0:D_HID, 14:15]
            g2_sb = bpack_sb[0:D_HID, 15:16]
            b2_sb = bpack_sb[0:D_HID, 16:17]
            oner_sb = rpack_sb[:, 0:PCH]
            onesbt_sb = rpack_sb[:, PCH : PCH + BT]
            eps_sb = wp.tile([PCH, 1], F32, name="eps_sb")
            nc.vector.memset(eps_sb[:, :], BN_EPS)

            # ncfw warm-up: first collective pays ~10us trigger latency, so
            # fire a tiny AllReduce early; it overlaps pass A compute.
            cwu_i = dp.tile([1, 2], F32, name="cwu_i")
            cwu_o = dp.tile([1, 2], F32, name="cwu_o")
            nc.gpsimd.dma_start(out=cwu_i[:, :], in_=rpack[:, 0:4].bitcast(F32))
            nc.gpsimd.collective_compute(
                "AllReduce",
                ALU.add,
                replica_groups=[list(range(NCORES))],
                ins=[cwu_i[:, :].opt()],
                outs=[cwu_o[:, :].opt()],
            )

            # ---- residents ----
            t1_res = [
                rp.tile([PCH, BS], BF16, name=f"t1_res_{m}") for m in range(_DMID_CH)
            ]
            t2_res = rp.tile([D_HID, BS], F32, name="t2_res")
            stats1 = [
                rp.tile([PCH, NT * 6], F32, name=f"stats1_{m}")
                for m in range(_DMID_CH)
            ]
            stats2 = rp.tile([D_HID, NT * 6], F32, name="stats2")

            # =========================== PASS A ===========================
            # Stages per tile:
            #   s1: batched x-load, L1 matmuls, relu           (PE + ACT:Relu)
            #   s2: L2 matmuls, exp, column-sum S              (PE + ACT:Exp)
            #   s3: rS = 1/S                                   (ACT:Recip)
            #   s4: broadcast, attn, wf, encoder-L1, BN stats  (PE + DVE)
            # Emission is software-pipelined by group: s1/s2 of group g+1 are
            # emitted before s3/s4 of group g, so the PE always has dense
            # matmul work while ACT switches LUTs.
            state = {}

            def a_s12(t):
                ts = slice(t * BT, (t + 1) * BT)
                xt = tp.tile([PCH, _DIN_CH, BT], BF16, name="xt", tag="xt",
                             bufs=2 * GRP)
                nc.sync.dma_start(
                    out=xt[:, :, :],
                    in_=xT[:, ts].rearrange("(c p) b -> p c b", c=_DIN_CH),
                )
                aT = []
                for m in range(_DIN_CH):
                    ps = pmm.tile([PCH, BT], F32, name="psa", tag="mm")
                    for kc in range(_DIN_CH):
                        nc.tensor.matmul(
                            ps[:, :],
                            aw1_sb[kc][:, m * PCH : (m + 1) * PCH],
                            xt[:, kc, :],
                            start=(kc == 0),
                            stop=(kc == _DIN_CH - 1),
                        )
                    am = tp.tile([PCH, BT], BF16, name="aT", tag=f"aT{m}",
                                 bufs=GRP + 1)
                    nc.scalar.activation(am[:, :], ps[:, :], AF.Relu, bias=ab1_sb[m])
                    aT.append(am)
                ee = []
                ps_s = ps1.tile([1, BT], F32, name="ps_s", tag="s1", bufs=3)
                for m in range(_DIN_CH):
                    ps = pmm.tile([PCH, BT], F32, name="psl", tag="mm")
                    for kc in range(_DIN_CH):
                        nc.tensor.matmul(
                            ps[:, :],
                            aw2_sb[kc][:, m * PCH : (m + 1) * PCH],
                            aT[kc][:, :],
                            start=(kc == 0),
                            stop=(kc == _DIN_CH - 1),
                        )
                    em = tp.tile([PCH, BT], BF16, name="ee", tag=f"ee{m}",
                                 bufs=2 * GRP - 1)
                    # exp without max-subtraction: logits are O(1) here
                    nc.scalar.activation(em[:, :], ps[:, :], AF.Exp, bias=ab2_sb[m])
                    ee.append(em)
                for m in range(_DIN_CH):
                    nc.tensor.matmul(
                        ps_s[:, :],
                        onec_sb[:, :],
                        ee[m][:, :],
                        start=(m == 0),
                        stop=(m == _DIN_CH - 1),
                    )
                state[t] = (xt, ee, ps_s)

            def a_s3(t):
                xt, ee, ps_s = state[t]
                rs = tp.tile([1, BT], BF16, name="rs", tag="rs", bufs=GRP + 1)
                _recip(nc, rs[:, :], ps_s[:, :])
                state[t] = (xt, ee, rs)

            def a_s4(t):
                ts = slice(t * BT, (t + 1) * BT)
                xt, ee, rs = state.pop(t)
                ps_rb = prb.tile([PCH, BT], F32, name="ps_rb", tag="rb")
                nc.tensor.matmul(
                    ps_rb[:, :], oner_sb[:, :], rs[:, :], start=True, stop=True
                )
                rb = tp.tile([PCH, BT], BF16, name="rb", tag="rb", bufs=2)
                nc.vector.tensor_copy(rb[:, :], ps_rb[:, :])
                at4 = tp.tile([PCH, _DIN_CH, BT], BF16, name="attn", tag="attn",
                              bufs=2)
                wf = []
                for m in range(_DIN_CH):
                    nc.vector.tensor_tensor(
                        at4[:, m, :], ee[m][:, :], rb[:, :], ALU.mult
                    )
                    wm = tp.tile([PCH, BT], BF16, name="wf", tag=f"wf{m}")
                    nc.vector.tensor_tensor(
                        wm[:, :], at4[:, m, :], xt[:, m, :], ALU.mult
                    )
                    wf.append(wm)
                nc.sync.dma_start(
                    out=attnT[:, ts].rearrange("(c p) b -> p c b", c=_DIN_CH),
                    in_=at4[:, :, :],
                )
                for m in range(_DMID_CH):
                    ps = pmm.tile([PCH, BT], F32, name="pst1", tag="mm")
                    for kc in range(_DIN_CH):
                        nc.tensor.matmul(
                            ps[:, :],
                            ew1_sb[kc][:, m * PCH : (m + 1) * PCH],
                            wf[kc][:, :],
                            start=(kc == 0),
                            stop=(kc == _DIN_CH - 1),
                        )
                    t1s = t1_res[m][:, ts]
                    nc.vector.tensor_scalar(
                        t1s, ps[:, :], eb1_sb[m][:, :], None, ALU.add
                    )
                    nc.vector.bn_stats(stats1[m][:, t * 6 : (t + 1) * 6], t1s)

            groups = [range(g, min(g + GRP, NT)) for g in range(0, NT, GRP)]
            prev = None
            for grp in groups:
                for t in grp:
                    a_s12(t)
                if prev is not None:
                    for t in prev:
                        a_s3(t)
                    for t in prev:
                        a_s4(t)
                prev = grp
            for t in prev:
                a_s3(t)
            for t in prev:
                a_s4(t)

            # ================== AllReduce BN1 statistics ==================
            # payload per chunk: [mean, E[x^2]] = [mean, var + mean^2]
            pay1 = []
            for m in range(_DMID_CH):
                agg = tp.tile([PCH, 2], F32, name="agg1", tag=f"agg1{m}", bufs=1)
                nc.vector.bn_aggr(agg[:, :], stats1[m][:, :])
                nc.vector.scalar_tensor_tensor(
                    agg[:, 1:2], agg[:, 0:1], agg[:, 0:1], agg[:, 1:2],
                    ALU.mult, ALU.add,
                )
                pay1.append(agg)
            cin1 = dp.tile([PCH, 2 * _DMID_CH], F32, name="cin1")
            cout1 = dp.tile([PCH, 2 * _DMID_CH], F32, name="cout1")
            for m in range(_DMID_CH):
                nc.gpsimd.dma_start(
                    out=cin1[:, 2 * m : 2 * m + 2], in_=pay1[m][:, :]
                )
            nc.gpsimd.collective_compute(
                "AllReduce",
                ALU.add,
                replica_groups=[list(range(NCORES))],
                ins=[cin1[:, :].opt()],
                outs=[cout1[:, :].opt()],
            )

            scale1, shift1 = [], []
            for m in range(_DMID_CH):
                gs = tp.tile([PCH, 2], F32, name="gs1", tag=f"gs1{m}", bufs=1)
                nc.gpsimd.dma_start(out=gs[:, :], in_=cout1[:, 2 * m : 2 * m + 2])
                nc.vector.tensor_scalar(gs[:, :], gs[:, :], 1.0 / NCORES, None, ALU.mult)
                var = tp.tile([PCH, 1], F32, name="var1", tag=f"var1{m}", bufs=1)
                # (mean*mean) - msq = -var ; Sqrt activation flips the sign
                nc.vector.scalar_tensor_tensor(
                    var[:, :], gs[:, 0:1], gs[:, 0:1], gs[:, 1:2],
                    ALU.mult, ALU.subtract,
                )
                std = tp.tile([PCH, 1], F32, name="std1", tag=f"std1{m}", bufs=1)
                nc.scalar.activation(
                    std[:, :], var[:, :], AF.Sqrt, bias=eps_sb[:, :], scale=-1.0
                )
                rstd = tp.tile([PCH, 1], F32, name="rstd1", tag=f"rstd1{m}", bufs=1)
                nc.vector.reciprocal(rstd[:, :], std[:, :])
                sc = tp.tile([PCH, 1], F32, name="sc1", tag=f"sc1{m}", bufs=1)
                nc.vector.tensor_tensor(sc[:, :], g1_sb[m][:, :], rstd[:, :], ALU.mult)
                sh = tp.tile([PCH, 1], F32, name="sh1", tag=f"sh1{m}", bufs=1)
                nc.vector.scalar_tensor_tensor(
                    sh[:, :], gs[:, 0:1], sc[:, :], b1_sb[m][:, :],
                    ALU.mult, ALU.subtract,
                )
                nc.vector.tensor_scalar(sh[:, :], sh[:, :], -1.0, None, ALU.mult)
                scale1.append(sc)
                shift1.append(sh)

            # =========================== PASS B ===========================
            for t in range(NT):
                ts = slice(t * BT, (t + 1) * BT)
                h1 = []
                for m in range(_DMID_CH):
                    hm = tp.tile([PCH, BT], F32R, name="h1", tag=f"h1{m}")
                    if m == 0:
                        nc.scalar.activation(
                            hm[:, :], t1_res[m][:, ts], AF.Relu,
                            bias=shift1[m][:, :], scale=scale1[m][:, :],
                        )
                    else:
                        nc.vector.tensor_scalar(
                            hm[:, :], t1_res[m][:, ts],
                            scale1[m][:, :], shift1[m][:, :], ALU.mult, ALU.add,
                        )
                        nc.vector.tensor_scalar(
                            hm[:, :], hm[:, :], 0.0, None, ALU.max
                        )
                    h1.append(hm)
                ps = pmm.tile([D_HID, BT], F32, name="pst2", tag="mm")
                for kc in range(_DMID_CH):
                    nc.tensor.matmul(
                        ps[:, :],
                        ew2r_sb[:, kc, :],
                        h1[kc][:, :],
                        start=(kc == 0),
                        stop=(kc == _DMID_CH - 1),
                    )
                t2s = t2_res[:, ts]
                nc.vector.tensor_scalar(t2s, ps[:, :], eb2_sb[:, :], None, ALU.add)
                nc.vector.bn_stats(stats2[:, t * 6 : (t + 1) * 6], t2s)

            # ================== AllReduce BN2 statistics ==================
            agg2 = tp.tile([D_HID, 2], F32, name="agg2", tag="agg2", bufs=1)
            nc.vector.bn_aggr(agg2[:, :], stats2[:, :])
            nc.vector.scalar_tensor_tensor(
                agg2[:, 1:2], agg2[:, 0:1], agg2[:, 0:1], agg2[:, 1:2],
                ALU.mult, ALU.add,
            )
            cin2 = dp.tile([D_HID, 2], F32, name="cin2")
            cout2 = dp.tile([D_HID, 2], F32, name="cout2")
            nc.gpsimd.dma_start(out=cin2[:, :], in_=agg2[:, :])
            nc.gpsimd.collective_compute(
                "AllReduce",
                ALU.add,
                replica_groups=[list(range(NCORES))],
                ins=[cin2[:, :].opt()],
                outs=[cout2[:, :].opt()],
            )
            gs2 = tp.tile([D_HID, 2], F32, name="gs2", tag="gs2", bufs=1)
            nc.gpsimd.dma_start(out=gs2[:, :], in_=cout2[:, :])
            nc.vector.tensor_scalar(gs2[:, :], gs2[:, :], 1.0 / NCORES, None, ALU.mult)
            var2 = tp.tile([D_HID, 1], F32, name="var2", tag="var2", bufs=1)
            nc.vector.scalar_tensor_tensor(
                var2[:, :], gs2[:, 0:1], gs2[:, 0:1], gs2[:, 1:2],
                ALU.mult, ALU.subtract,
            )
            std2 = tp.tile([D_HID, 1], F32, name="std2", tag="std2", bufs=1)
            nc.scalar.activation(
                std2[:, :], var2[:, :], AF.Sqrt, bias=eps_sb[0:D_HID, :], scale=-1.0
            )
            rstd2 = tp.tile([D_HID, 1], F32, name="rstd2", tag="rstd2", bufs=1)
            nc.vector.reciprocal(rstd2[:, :], std2[:, :])
            scale2 = tp.tile([D_HID, 1], F32, name="scale2", tag="scale2", bufs=1)
            nc.vector.tensor_tensor(scale2[:, :], g2_sb[:, :], rstd2[:, :], ALU.mult)
            shift2 = tp.tile([D_HID, 1], F32, name="shift2", tag="shift2", bufs=1)
            nc.vector.scalar_tensor_tensor(
                shift2[:, :], gs2[:, 0:1], scale2[:, :], b2_sb[:, :],
                ALU.mult, ALU.subtract,
            )
            nc.vector.tensor_scalar(shift2[:, :], shift2[:, :], -1.0, None, ALU.mult)

            # =========================== PASS C ===========================
            # rhs_aug rows: [enc (64); enorm (1); ones (1)];
            # caug = [-2*C^T ; ones row ; ||c||^2 row]  so one matmul gives
            # d2 = ||enc||^2 + ||c||^2 - 2 enc.c . Membership = (1/d2)
            # normalized over clusters (m_fuzz=2 -> power 2; the reference's
            # sqrt + 1e-10 clamp cancels; d2 >> 1e-20 for this data).
            cstate = {}

            def c_s1(t):
                ts = slice(t * BT, (t + 1) * BT)
                aug = tp.tile([D_HID + 2, BT], BF16, name="aug", tag="aug",
                              bufs=2 * GRP - 1)
                nc.gpsimd.dma_start(out=aug[D_HID + 1 : D_HID + 2, :], in_=rpack[:, PCH : PCH + BT])
                enc = aug[0:D_HID, :]
                nc.scalar.activation(
                    enc, t2_res[:, ts], AF.Tanh,
                    bias=shift2[:, :], scale=scale2[:, :],
                )
                nc.gpsimd.dma_start(out=encT[:, ts], in_=enc)
                esq = tp.tile([D_HID, BT], BF16, name="esq", tag="esq", bufs=3)
                nc.vector.tensor_tensor(
                    esq[:, :], aug[0:D_HID, :], aug[0:D_HID, :], ALU.mult
                )
                ps_en = ps1.tile([1, BT], F32, name="ps_en", tag="s1", bufs=3)
                nc.tensor.matmul(
                    ps_en[:, :], onec_sb[0:D_HID, :], esq[:, :],
                    start=True, stop=True,
                )
                nc.vector.tensor_copy(aug[D_HID : D_HID + 1, :], ps_en[:, :])
                ps_d2 = pmm.tile([K, BT], F32, name="ps_d2", tag="mm")
                nc.tensor.matmul(
                    ps_d2[:, :], caug_sb[:, :], aug[:, :], start=True, stop=True
                )
                cstate[t] = ps_d2

            def c_s2(t):
                ps_d2 = cstate[t]
                inv = tp.tile([K, BT], BF16, name="inv", tag="inv", bufs=GRP + 1)
                _recip(nc, inv[:, :], ps_d2[:, :])
                ps_s2 = ps1.tile([1, BT], F32, name="ps_s2", tag="s1", bufs=3)
                nc.tensor.matmul(
                    ps_s2[:, :], onec_sb[0:K, :], inv[:, :], start=True, stop=True
                )
                rs2 = tp.tile([1, BT], BF16, name="rs2", tag="rs", bufs=GRP + 1)
                _recip(nc, rs2[:, :], ps_s2[:, :])
                cstate[t] = (inv, rs2)

            def c_s3(t):
                ts = slice(t * BT, (t + 1) * BT)
                inv, rs2 = cstate.pop(t)
                ps_rb2 = prb.tile([K, BT], F32, name="ps_rb2", tag="rb")
                nc.tensor.matmul(
                    ps_rb2[:, :], oner_sb[:, 0:K], rs2[:, :], start=True, stop=True
                )
                mb = tp.tile([K, BT], BF16, name="mb", tag="mb", bufs=3)
                nc.vector.tensor_tensor(
                    mb[:, :], inv[:, :], ps_rb2[:, :], ALU.mult
                )
                nc.gpsimd.dma_start(out=membT[:, ts], in_=mb[:, :])

            prev = None
            for grp in groups:
                for t in grp:
                    c_s1(t)
                if prev is not None:
                    for t in prev:
                        c_s2(t)
                    for t in prev:
                        c_s3(t)
                prev = grp
            for t in prev:
                c_s2(t)
            for t in prev:
                c_s3(t)

    _split_excess_waits(nc)
    return nc


_built = None


def _get_nc():
    global _built
    if _built is None:
        _built = build()
    return _built


def _prep_in_maps(inputs):
    import ml_dtypes

    f32 = np.float32
    bf16 = ml_dtypes.bfloat16
    x = np.asarray(inputs["x"], f32)
    xT_full = np.ascontiguousarray(x.T.astype(bf16))  # [512, 65536]

    centers = np.asarray(inputs["centers"], f32)
    cnorm = (centers.astype(np.float64) ** 2).sum(axis=1).astype(f32)
    caug = np.concatenate(
        [-2.0 * centers.T, np.ones((1, K), f32), cnorm[None, :]], axis=0
    )

    def bf(v):
        return np.asarray(v, f32).astype(bf16)

    wpack = np.concatenate(
        [bf(inputs["aw1"]).reshape(4, PCH, D_IN).transpose(1, 0, 2).reshape(PCH, -1),
         bf(inputs["aw2"]).reshape(4, PCH, D_IN).transpose(1, 0, 2).reshape(PCH, -1),
         bf(inputs["ew1"]).reshape(4, PCH, D_MID).transpose(1, 0, 2).reshape(PCH, -1),
         bf(inputs["ew2"]).reshape(2, PCH, D_HID).transpose(1, 0, 2).reshape(PCH, -1),
         np.ones((PCH, 1), bf16)],
        axis=1,
    )
    bpack = np.zeros((PCH, 17), f32)
    bpack[:, 0:4] = np.asarray(inputs["ab1"], f32).reshape(4, PCH).T
    bpack[:, 4:8] = np.asarray(inputs["ab2"], f32).reshape(4, PCH).T
    bpack[:, 8:10] = np.asarray(inputs["eb1"], f32).reshape(2, PCH).T
    bpack[:, 10:12] = np.asarray(inputs["g1"], f32).reshape(2, PCH).T
    bpack[:, 12:14] = np.asarray(inputs["b1"], f32).reshape(2, PCH).T
    bpack[0:D_HID, 14] = np.asarray(inputs["eb2"], f32)
    bpack[0:D_HID, 15] = np.asarray(inputs["g2"], f32)
    bpack[0:D_HID, 16] = np.asarray(inputs["b2"], f32)
    rpack = np.ones((1, PCH + BT), bf16)

    shared = {
        "wpack": np.ascontiguousarray(wpack),
        "bpack": np.ascontiguousarray(bpack),
        "caug": np.ascontiguousarray(bf(caug)),
        "ew2r": np.ascontiguousarray(np.asarray(inputs["ew2"], f32)),
        "rpack": np.ascontiguousarray(rpack),
    }
    in_maps = []
    for c in range(NCORES):
        m = dict(shared)
        m["xT"] = np.ascontiguousarray(xT_full[:, c * BS : (c + 1) * BS])
        in_maps.append(m)
    return in_maps


def _run(inputs, trace=False):
    nc = _get_nc()
    in_maps = _prep_in_maps(inputs)
    res = run_bass_kernel_spmd(
        nc, in_maps, core_ids=list(range(NCORES)), trace=trace
    )
    attn = np.empty((B, D_IN), np.float32)
    enc = np.empty((B, D_HID), np.float32)
    memb = np.empty((B, D_HID), np.float32)
    for c in range(NCORES):
        r = res.results[c]
        sl = slice(c * BS, (c + 1) * BS)
        attn[sl, :] = np.asarray(r["attnT"]).astype(np.float32).T
        enc[sl, :] = np.asarray(r["encT"]).astype(np.float32).T
        memb[sl, :] = np.asarray(r["membT"]).astype(np.float32).T
    return (attn, enc, memb), res


def kernel(**inputs):
    out, _ = _run(inputs, trace=False)
    return out


# revision 25
# speedup vs baseline: 1.0424x; 1.0424x over previous
"""Trainium2 Bass kernel for nn_AFDEKM_Model (attention-gated encoder + fuzzy
c-means membership), data-parallel over 8 NeuronCores.

Layout: feature-major on device — activations live as [features, batch_tile]
so Linear layers need no transposes (weights [in, out] are already the lhsT
the TensorEngine wants), BatchNorm batch-reductions are free-axis DVE ops, and
BN apply is one fused ScalarEngine activation per tile. Softmax over features
(partition axis) uses ones-matmul column sums plus a K=1 broadcast matmul.
The host transposes x once and transposes the three outputs back.

Matmul operands are bf16 (full-rate PE path; measured 216ns/MM warm at N=512),
PSUM accumulation fp32. Tiles are processed in groups with software-pipelined
emission (group g's normalize/encode stage is emitted after group g+1's
matmul stages) so the TensorEngine never idles long enough for the HAM clock
gate to re-throttle it, and the ScalarEngine runs same-LUT activations
back-to-back (table reloads cost ~1.3us). Training-mode BatchNorm over the
full 65536-row batch: per-core partial stats are combined with two tiny
AllReduces.
"""

import sys
import types

import numpy as np

B, D_IN, D_MID, D_HID, K = 65536, 512, 256, 64, 64
BN_EPS = 1e-5
NCORES = 8
BS = B // NCORES          # 8192 rows per core
BT = 512                  # batch tile (free dim; == PSUM bank / BN_STATS_FMAX)
NT = BS // BT             # 16 tiles per core
GRP = 4                   # tiles per ACT-table group
PCH = 128                 # partition chunk

_DIN_CH = D_IN // PCH     # 4
_DMID_CH = D_MID // PCH   # 2


def _shim_antenv_hooks():
    # this container's antenv lacks axon_hooks; bass_utils imports it when
    # trace=True. Harmless no-op module unless test.py installs the real one.
    if "antenv.axon_hooks" not in sys.modules:
        mod = types.ModuleType("antenv.axon_hooks")
        holder = {}
        mod.set_axon_ntff_profile_hook = lambda h: holder.__setitem__("h", h)
        mod.get_axon_ntff_profile_hook = lambda: holder.get("h")
        sys.modules["antenv.axon_hooks"] = mod


_shim_antenv_hooks()

import concourse.mybir as mybir  # noqa: E402
import concourse.bass as bass  # noqa: E402
import concourse.tile as tile  # noqa: E402
from concourse.bass_utils import run_bass_kernel_spmd  # noqa: E402

F32 = mybir.dt.float32
BF16 = mybir.dt.bfloat16
AF = mybir.ActivationFunctionType
ALU = mybir.AluOpType

_MAX_WAITS = 1  # this neuronxcc build allows one sync-wait per instruction
_wait_ctr = [0]


def _split_excess_waits(nc):
    """Spill excess semaphore waits onto same-engine NoOps (walrus here
    rejects instructions with >1 sync wait)."""
    for fn in nc.m.functions:
        for bb in fn.blocks:
            idx = 0
            while idx < len(bb.instructions):
                ins = bb.instructions[idx]
                si = ins.sync_info
                if si is None:
                    idx += 1
                    continue
                waits = list(si.on_wait)
                if len(waits) <= _MAX_WAITS:
                    idx += 1
                    continue
                keep = waits[-_MAX_WAITS:]
                extra = waits[:-_MAX_WAITS]
                n_ins = 0
                for i in range(0, len(extra), _MAX_WAITS):
                    chunk = extra[i : i + _MAX_WAITS]
                    _wait_ctr[0] += 1
                    nop = mybir.InstNoOp(
                        name=f"{ins.name}-wspill{_wait_ctr[0]}",
                        sync_info=mybir.SyncInfo(on_wait=chunk, on_update=[]),
                        bass_nofuse=True,
                        engine=ins.engine,
                    )
                    nc.register_instruction(nop, overwrite=True)
                    bb.instructions.insert(idx + n_ins, nop)
                    n_ins += 1
                ins.sync_info = mybir.SyncInfo(
                    on_wait=keep, on_update=list(si.on_update)
                )
                idx += n_ins + 1


def _recip(nc, out_ap, in_ap):
    """1/x on the ScalarEngine LUT (HW-measured ~1e-5 max rel err over
    1e-2..1e6 — plenty for softmax/membership normalizers). bass's
    activation() refuses Reciprocal wholesale, so emit InstActivation
    directly."""
    eng = nc.scalar
    inputs = [eng.lower_ap(in_ap)]
    for arg in (0.0, 1.0, 0.0):  # bias, scale, alpha
        inputs.append(mybir.ImmediateValue(dtype=mybir.dt.float32, value=float(arg)))
    return eng.add_instruction(
        mybir.InstActivation(
            name=eng.bass.get_next_instruction_name(),
            func=AF.Reciprocal,
            ins=inputs,
            outs=[eng.lower_ap(out_ap)],
        )
    )


def build():
    nc = bass.Bass()

    # ---- I/O ----
    xT = nc.declare_dram_parameter("xT", [D_IN, BS], BF16, isOutput=False)
    # all [128, *] weights packed into one tensor -> one startup DMA
    # cols: aw1 (4x512) | aw2 (4x512) | ew1 (4x256) | ew2 (2x64) | ones_col (1)
    WPACK = 4 * D_IN + 4 * D_IN + 4 * D_MID + 2 * D_HID + 1
    wpack = nc.declare_dram_parameter("wpack", [PCH, WPACK], BF16, isOutput=False)
    # biases/gammas packed: ab1 (4) | ab2 (4) | eb1 (2) | g1 (2) | b1 (2) |
    # eb2 | g2 | b2  (last three on partitions 0:64)
    bpack = nc.declare_dram_parameter("bpack", [PCH, 17], F32, isOutput=False)
    caug = nc.declare_dram_parameter("caug", [D_HID + 2, K], BF16, isOutput=False)
    # ones_row (128) | ones_bt (512) on one partition
    rpack = nc.declare_dram_parameter("rpack", [1, PCH + BT], BF16, isOutput=False)
    # encoder layer-2 weights in float32r: the e2 matmul is tiny (2 per tile)
    # and running it at full fp32 precision halves enc's worst-case error
    F32R = mybir.dt.float32r
    ew2r = nc.declare_dram_parameter("ew2r", [D_MID, D_HID], F32R, isOutput=False)

    attnT = nc.declare_dram_parameter("attnT", [D_IN, BS], BF16, isOutput=True)
    encT = nc.declare_dram_parameter("encT", [D_HID, BS], BF16, isOutput=True)
    membT = nc.declare_dram_parameter("membT", [D_HID, BS], BF16, isOutput=True)

    with tile.TileContext(nc) as tc:
        with (
            tc.tile_pool(name="wp", bufs=1) as wp,
            tc.tile_pool(name="rp", bufs=1) as rp,
            tc.tile_pool(name="tp", bufs=2) as tp,
            tc.tile_pool(name="pmm", bufs=4, space="PSUM") as pmm,
            tc.tile_pool(name="ps1", bufs=4, space="PSUM") as ps1,
            tc.tile_pool(name="prb", bufs=1, space="PSUM") as prb,
            tc.tile_pool(name="dp", bufs=1, space="DRAM") as dp,
        ):
            # ---- load weights / constants (one DMA each) ----
            wpack_sb = wp.tile([PCH, 4 * D_IN + 4 * D_IN + 4 * D_MID + 2 * D_HID + 1], BF16, name="wpack_sb")
            nc.sync.dma_start(out=wpack_sb[:, :], in_=wpack[:, :])
            bpack_sb = wp.tile([PCH, 17], F32, name="bpack_sb")
            nc.sync.dma_start(out=bpack_sb[:, :], in_=bpack[:, :])
            caug_sb = wp.tile([D_HID + 2, K], BF16, name="caug_sb")
            nc.sync.dma_start(out=caug_sb[:, :], in_=caug[:, :])
            rpack_sb = wp.tile([1, PCH + BT], BF16, name="rpack_sb")
            nc.sync.dma_start(out=rpack_sb[:, :], in_=rpack[:, :])
            ew2r_sb = wp.tile([PCH, _DMID_CH, D_HID], F32R, name="ew2r_sb")
            nc.sync.dma_start(
                out=ew2r_sb[:, :, :],
                in_=ew2r[:, :].rearrange("(c p) h -> p c h", c=_DMID_CH),
            )

            o = [0]

            def _wslice(n):
                a = wpack_sb[:, o[0] : o[0] + n]
                o[0] += n
                return a

            aw1_sb = [_wslice(D_IN) for _ in range(_DIN_CH)]
            aw2_sb = [_wslice(D_IN) for _ in range(_DIN_CH)]
            ew1_sb = [_wslice(D_MID) for _ in range(_DIN_CH)]
            ew2_sb = [_wslice(D_HID) for _ in range(_DMID_CH)]
            onec_sb = _wslice(1)
            ab1_sb = [bpack_sb[:, m : m + 1] for m in range(4)]
            ab2_sb = [bpack_sb[:, 4 + m : 5 + m] for m in range(4)]
            eb1_sb = [bpack_sb[:, 8 + m : 9 + m] for m in range(2)]
            g1_sb = [bpack_sb[:, 10 + m : 11 + m] for m in range(2)]
            b1_sb = [bpack_sb[:, 12 + m : 13 + m] for m in range(2)]
            eb2_sb = bpack_sb[0:D_HID, 14:15]
            g2_sb = bpack_sb[0:D_HID, 15:16]
            b2_sb = bpack_sb[0:D_HID, 16:17]
            oner_sb = rpack_sb[:, 0:PCH]
            onesbt_sb = rpack_sb[:, PCH : PCH + BT]
            eps_sb = wp.tile([PCH, 1], F32, name="eps_sb")
            nc.vector.memset(eps_sb[:, :], BN_EPS)

            # ncfw warm-up: first collective pays ~10us trigger latency, so
            # fire a tiny AllReduce early; it overlaps pass A compute.
            cwu_i = dp.tile([1, 2], F32, name="cwu_i")
            cwu_o = dp.tile([1, 2], F32, name="cwu_o")
            nc.gpsimd.dma_start(out=cwu_i[:, :], in_=rpack[:, 0:4].bitcast(F32))
            nc.gpsimd.collective_compute(
                "AllReduce",
                ALU.add,
                replica_groups=[list(range(NCORES))],
                ins=[cwu_i[:, :].opt()],
                outs=[cwu_o[:, :].opt()],
            )

            # ---- residents ----
            t1_res = [
                rp.tile([PCH, BS], BF16, name=f"t1_res_{m}") for m in range(_DMID_CH)
            ]
            t2_res = rp.tile([D_HID, BS], F32, name="t2_res")
            stats1 = [
                rp.tile([PCH, NT * 6], F32, name=f"stats1_{m}")
                for m in range(_DMID_CH)
            ]
            stats2 = rp.tile([D_HID, NT * 6], F32, name="stats2")

            # =========================== PASS A ===========================
            # Stages per tile:
            #   s1: batched x-load, L1 matmuls, relu           (PE + ACT:Relu)
            #   s2: L2 matmuls, exp, column-sum S              (PE + ACT:Exp)
            #   s3: rS = 1/S                                   (ACT:Recip)
            #   s4: broadcast, attn, wf, encoder-L1, BN stats  (PE + DVE)
            # Emission is software-pipelined by group: s1/s2 of group g+1 are
            # emitted before s3/s4 of group g, so the PE always has dense
            # matmul work while ACT switches LUTs.
            state = {}

            def a_s12(t):
                ts = slice(t * BT, (t + 1) * BT)
                xt = tp.tile([PCH, _DIN_CH, BT], BF16, name="xt", tag="xt",
                             bufs=2 * GRP)
                nc.sync.dma_start(
                    out=xt[:, :, :],
                    in_=xT[:, ts].rearrange("(c p) b -> p c b", c=_DIN_CH),
                )
                aT = []
                for m in range(_DIN_CH):
                    ps = pmm.tile([PCH, BT], F32, name="psa", tag="mm")
                    for kc in range(_DIN_CH):
                        nc.tensor.matmul(
                            ps[:, :],
                            aw1_sb[kc][:, m * PCH : (m + 1) * PCH],
                            xt[:, kc, :],
                            start=(kc == 0),
                            stop=(kc == _DIN_CH - 1),
                        )
                    am = tp.tile([PCH, BT], BF16, name="aT", tag=f"aT{m}",
                                 bufs=GRP + 1)
                    nc.scalar.activation(am[:, :], ps[:, :], AF.Relu, bias=ab1_sb[m])
                    aT.append(am)
                ee = []
                ps_s = ps1.tile([1, BT], F32, name="ps_s", tag="s1", bufs=3)
                for m in range(_DIN_CH):
                    ps = pmm.tile([PCH, BT], F32, name="psl", tag="mm")
                    for kc in range(_DIN_CH):
                        nc.tensor.matmul(
                            ps[:, :],
                            aw2_sb[kc][:, m * PCH : (m + 1) * PCH],
                            aT[kc][:, :],
                            start=(kc == 0),
                            stop=(kc == _DIN_CH - 1),
                        )
                    em = tp.tile([PCH, BT], BF16, name="ee", tag=f"ee{m}",
                                 bufs=2 * GRP - 1)
                    # exp without max-subtraction: logits are O(1) here
                    nc.scalar.activation(em[:, :], ps[:, :], AF.Exp, bias=ab2_sb[m])
                    ee.append(em)
                for m in range(_DIN_CH):
                    nc.tensor.matmul(
                        ps_s[:, :],
                        onec_sb[:, :],
                        ee[m][:, :],
                        start=(m == 0),
                        stop=(m == _DIN_CH - 1),
                    )
                state[t] = (xt, ee, ps_s)

            def a_s3(t):
                xt, ee, ps_s = state[t]
                rs = tp.tile([1, BT], BF16, name="rs", tag="rs", bufs=GRP + 1)
                _recip(nc, rs[:, :], ps_s[:, :])
                state[t] = (xt, ee, rs)

            def a_s4(t):
                ts = slice(t * BT, (t + 1) * BT)
                xt, ee, rs = state.pop(t)
                ps_rb = prb.tile([PCH, BT], F32, name="ps_rb", tag="rb")
                nc.tensor.matmul(
                    ps_rb[:, :], oner_sb[:, :], rs[:, :], start=True, stop=True
                )
                rb = tp.tile([PCH, BT], BF16, name="rb", tag="rb", bufs=2)
                nc.vector.tensor_copy(rb[:, :], ps_rb[:, :])
                at4 = tp.tile([PCH, _DIN_CH, BT], BF16, name="attn", tag="attn",
                              bufs=2)
                wf = []
                for m in range(_DIN_CH):
                    nc.vector.tensor_tensor(
                        at4[:, m, :], ee[m][:, :], rb[:, :], ALU.mult
                    )
                    wm = tp.tile([PCH, BT], BF16, name="wf", tag=f"wf{m}")
                    nc.vector.tensor_tensor(
                        wm[:, :], at4[:, m, :], xt[:, m, :], ALU.mult
                    )
                    wf.append(wm)
                nc.sync.dma_start(
                    out=attnT[:, ts].rearrange("(c p) b -> p c b", c=_DIN_CH),
                    in_=at4[:, :, :],
                )
                for m in range(_DMID_CH):
                    ps = pmm.tile([PCH, BT], F32, name="pst1", tag="mm")
                    for kc in range(_DIN_CH):
                        nc.tensor.matmul(
                            ps[:, :],
                            ew1_sb[kc][:, m * PCH : (m + 1) * PCH],
                            wf[kc][:, :],
                            start=(kc == 0),
                            stop=(kc == _DIN_CH - 1),
                        )
                    t1s = t1_res[m][:, ts]
                    nc.vector.tensor_scalar(
                        t1s, ps[:, :], eb1_sb[m][:, :], None, ALU.add
                    )
                    nc.vector.bn_stats(stats1[m][:, t * 6 : (t + 1) * 6], t1s)

            groups = [range(g, min(g + GRP, NT)) for g in range(0, NT, GRP)]
            prev = None
            for grp in groups:
                for t in grp:
                    a_s12(t)
                if prev is not None:
                    for t in prev:
                        a_s3(t)
                    for t in prev:
                        a_s4(t)
                prev = grp
            for t in prev:
                a_s3(t)
            for t in prev:
                a_s4(t)

            # ================== AllReduce BN1 statistics ==================
            # payload per chunk: [mean, E[x^2]] = [mean, var + mean^2]
            pay1 = []
            for m in range(_DMID_CH):
                agg = tp.tile([PCH, 2], F32, name="agg1", tag=f"agg1{m}", bufs=1)
                nc.vector.bn_aggr(agg[:, :], stats1[m][:, :])
                nc.vector.scalar_tensor_tensor(
                    agg[:, 1:2], agg[:, 0:1], agg[:, 0:1], agg[:, 1:2],
                    ALU.mult, ALU.add,
                )
                pay1.append(agg)
            cin1 = dp.tile([PCH, 2 * _DMID_CH], F32, name="cin1")
            cout1 = dp.tile([PCH, 2 * _DMID_CH], F32, name="cout1")
            for m in range(_DMID_CH):
                nc.gpsimd.dma_start(
                    out=cin1[:, 2 * m : 2 * m + 2], in_=pay1[m][:, :]
                )
            nc.gpsimd.collective_compute(
                "AllReduce",
                ALU.add,
                replica_groups=[list(range(NCORES))],
                ins=[cin1[:, :].opt()],
                outs=[cout1[:, :].opt()],
            )

            scale1, shift1 = [], []
            for m in range(_DMID_CH):
                gs = tp.tile([PCH, 2], F32, name="gs1", tag=f"gs1{m}", bufs=1)
                nc.gpsimd.dma_start(out=gs[:, :], in_=cout1[:, 2 * m : 2 * m + 2])
                nc.vector.tensor_scalar(gs[:, :], gs[:, :], 1.0 / NCORES, None, ALU.mult)
                var = tp.tile([PCH, 1], F32, name="var1", tag=f"var1{m}", bufs=1)
                # (mean*mean) - msq = -var ; Sqrt activation flips the sign
                nc.vector.scalar_tensor_tensor(
                    var[:, :], gs[:, 0:1], gs[:, 0:1], gs[:, 1:2],
                    ALU.mult, ALU.subtract,
                )
                std = tp.tile([PCH, 1], F32, name="std1", tag=f"std1{m}", bufs=1)
                nc.scalar.activation(
                    std[:, :], var[:, :], AF.Sqrt, bias=eps_sb[:, :], scale=-1.0
                )
                rstd = tp.tile([PCH, 1], F32, name="rstd1", tag=f"rstd1{m}", bufs=1)
                nc.vector.reciprocal(rstd[:, :], std[:, :])
                sc = tp.tile([PCH, 1], F32, name="sc1", tag=f"sc1{m}", bufs=1)
                nc.vector.tensor_tensor(sc[:, :], g1_sb[m][:, :], rstd[:, :], ALU.mult)
                sh = tp.tile([PCH, 1], F32, name="sh1", tag=f"sh1{m}", bufs=1)
                nc.vector.scalar_tensor_tensor(
                    sh[:, :], gs[:, 0:1], sc[:, :], b1_sb[m][:, :],
                    ALU.mult, ALU.subtract,
                )
                nc.vector.tensor_scalar(sh[:, :], sh[:, :], -1.0, None, ALU.mult)
                scale1.append(sc)
                shift1.append(sh)

            # =========================== PASS B ===========================
            for t in range(NT):
                ts = slice(t * BT, (t + 1) * BT)
                h1 = []
                for m in range(_DMID_CH):
                    hm = tp.tile([PCH, BT], F32R, name="h1", tag=f"h1{m}")
                    nc.scalar.activation(
                        hm[:, :], t1_res[m][:, ts], AF.Relu,
                        bias=shift1[m][:, :], scale=scale1[m][:, :],
                    )
                    h1.append(hm)
                ps = pmm.tile([D_HID, BT], F32, name="pst2", tag="mm")
                for kc in range(_DMID_CH):
                    nc.tensor.matmul(
                        ps[:, :],
                        ew2r_sb[:, kc, :],
                        h1[kc][:, :],
                        start=(kc == 0),
                        stop=(kc == _DMID_CH - 1),
                    )
                t2s = t2_res[:, ts]
                nc.vector.tensor_scalar(t2s, ps[:, :], eb2_sb[:, :], None, ALU.add)
                nc.vector.bn_stats(stats2[:, t * 6 : (t + 1) * 6], t2s)

            # ================== AllReduce BN2 statistics ==================
            agg2 = tp.tile([D_HID, 2], F32, name="agg2", tag="agg2", bufs=1)
            nc.vector.bn_aggr(agg2[:, :], stats2[:, :])
            nc.vector.scalar_tensor_tensor(
                agg2[:, 1:2], agg2[:, 0:1], agg2[:, 0:1], agg2[:, 1:2],
                ALU.mult, ALU.add,
            )
            cin2 = dp.tile([D_HID, 2], F32, name="cin2")
            cout2 = dp.tile([D_HID, 2], F32, name="cout2")
            nc.gpsimd.dma_start(out=cin2[:, :], in_=agg2[:, :])
            nc.gpsimd.collective_compute(
                "AllReduce",
                ALU.add,
                replica_groups=[list(range(NCORES))],
                ins=[cin2[:, :].opt()],
                outs=[cout2[:, :].opt()],
            )
            gs2 = tp.tile([D_HID, 2], F32, name="gs2", tag="gs2", bufs=1)
            nc.gpsimd.dma_start(out=gs2[:, :], in_=cout2[:, :])
            nc.vector.tensor_scalar(gs2[:, :], gs2[:, :], 1.0 / NCORES, None, ALU.mult)
            var2 = tp.tile([D_HID, 1], F32, name="var2", tag="var2", bufs=1)
            nc.vector.scalar_tensor_tensor(
                var2[:, :], gs2[:, 0:1], gs2[:, 0:1], gs2[:, 1:2],
                ALU.mult, ALU.subtract,
            )
            std2 = tp.tile([D_HID, 1], F32, name="std2", tag="std2", bufs=1)
            nc.scalar.activation(
                std2[:, :], var2[:, :], AF.Sqrt, bias=eps_sb[0:D_HID, :], scale=-1.0
            )
            rstd2 = tp.tile([D_HID, 1], F32, name="rstd2", tag="rstd2", bufs=1)
            nc.vector.reciprocal(rstd2[:, :], std2[:, :])
            scale2 = tp.tile([D_HID, 1], F32, name="scale2", tag="scale2", bufs=1)
            nc.vector.tensor_tensor(scale2[:, :], g2_sb[:, :], rstd2[:, :], ALU.mult)
            shift2 = tp.tile([D_HID, 1], F32, name="shift2", tag="shift2", bufs=1)
            nc.vector.scalar_tensor_tensor(
                shift2[:, :], gs2[:, 0:1], scale2[:, :], b2_sb[:, :],
                ALU.mult, ALU.subtract,
            )
            nc.vector.tensor_scalar(shift2[:, :], shift2[:, :], -1.0, None, ALU.mult)

            # =========================== PASS C ===========================
            # rhs_aug rows: [enc (64); enorm (1); ones (1)];
            # caug = [-2*C^T ; ones row ; ||c||^2 row]  so one matmul gives
            # d2 = ||enc||^2 + ||c||^2 - 2 enc.c . Membership = (1/d2)
            # normalized over clusters (m_fuzz=2 -> power 2; the reference's
            # sqrt + 1e-10 clamp cancels; d2 >> 1e-20 for this data).
            cstate = {}

            def c_s1(t):
                ts = slice(t * BT, (t + 1) * BT)
                aug = tp.tile([D_HID + 2, BT], BF16, name="aug", tag="aug",
                              bufs=2 * GRP - 1)
                nc.gpsimd.dma_start(out=aug[D_HID + 1 : D_HID + 2, :], in_=rpack[:, PCH : PCH + BT])
                enc = aug[0:D_HID, :]
                nc.scalar.activation(
                    enc, t2_res[:, ts], AF.Tanh,
                    bias=shift2[:, :], scale=scale2[:, :],
                )
                nc.gpsimd.dma_start(out=encT[:, ts], in_=enc)
                esq = tp.tile([D_HID, BT], BF16, name="esq", tag="esq", bufs=3)
                nc.vector.tensor_tensor(
                    esq[:, :], aug[0:D_HID, :], aug[0:D_HID, :], ALU.mult
                )
                ps_en = ps1.tile([1, BT], F32, name="ps_en", tag="s1", bufs=3)
                nc.tensor.matmul(
                    ps_en[:, :], onec_sb[0:D_HID, :], esq[:, :],
                    start=True, stop=True,
                )
                nc.vector.tensor_copy(aug[D_HID : D_HID + 1, :], ps_en[:, :])
                ps_d2 = pmm.tile([K, BT], F32, name="ps_d2", tag="mm")
                nc.tensor.matmul(
                    ps_d2[:, :], caug_sb[:, :], aug[:, :], start=True, stop=True
                )
                cstate[t] = ps_d2

            def c_s2(t):
                ps_d2 = cstate[t]
                inv = tp.tile([K, BT], BF16, name="inv", tag="inv", bufs=GRP + 1)
                _recip(nc, inv[:, :], ps_d2[:, :])
                ps_s2 = ps1.tile([1, BT], F32, name="ps_s2", tag="s1", bufs=3)
                nc.tensor.matmul(
                    ps_s2[:, :], onec_sb[0:K, :], inv[:, :], start=True, stop=True
                )
                rs2 = tp.tile([1, BT], BF16, name="rs2", tag="rs", bufs=GRP + 1)
                _recip(nc, rs2[:, :], ps_s2[:, :])
                cstate[t] = (inv, rs2)

            def c_s3(t):
                ts = slice(t * BT, (t + 1) * BT)
                inv, rs2 = cstate.pop(t)
                ps_rb2 = prb.tile([K, BT], F32, name="ps_rb2", tag="rb")
                nc.tensor.matmul(
                    ps_rb2[:, :], oner_sb[:, 0:K], rs2[:, :], start=True, stop=True
                )
                mb = tp.tile([K, BT], BF16, name="mb", tag="mb", bufs=3)
                nc.vector.tensor_tensor(
                    mb[:, :], inv[:, :], ps_rb2[:, :], ALU.mult
                )
                nc.gpsimd.dma_start(out=membT[:, ts], in_=mb[:, :])

            prev = None
            for grp in groups:
                for t in grp:
                    c_s1(t)
                if prev is not None:
                    for t in prev:
                        c_s2(t)
                    for t in prev:
                        c_s3(t)
                prev = grp
            for t in prev:
                c_s2(t)
            for t in prev:
                c_s3(t)

    _split_excess_waits(nc)
    return nc


_built = None


def _get_nc():
    global _built
    if _built is None:
        _built = build()
    return _built


def _prep_in_maps(inputs):
    import ml_dtypes

    f32 = np.float32
    bf16 = ml_dtypes.bfloat16
    x = np.asarray(inputs["x"], f32)
    xT_full = np.ascontiguousarray(x.T.astype(bf16))  # [512, 65536]

    centers = np.asarray(inputs["centers"], f32)
    cnorm = (centers.astype(np.float64) ** 2).sum(axis=1).astype(f32)
    caug = np.concatenate(
        [-2.0 * centers.T, np.ones((1, K), f32), cnorm[None, :]], axis=0
    )

    def bf(v):
        return np.asarray(v, f32).astype(bf16)

    wpack = np.concatenate(
        [bf(inputs["aw1"]).reshape(4, PCH, D_IN).transpose(1, 0, 2).reshape(PCH, -1),
         bf(inputs["aw2"]).reshape(4, PCH, D_IN).transpose(1, 0, 2).reshape(PCH, -1),
         bf(inputs["ew1"]).reshape(4, PCH, D_MID).transpose(1, 0, 2).reshape(PCH, -1),
         bf(inputs["ew2"]).reshape(2, PCH, D_HID).transpose(1, 0, 2).reshape(PCH, -1),
         np.ones((PCH, 1), bf16)],
        axis=1,
    )
    bpack = np.zeros((PCH, 17), f32)
    bpack[:, 0:4] = np.asarray(inputs["ab1"], f32).reshape(4, PCH).T
    bpack[:, 4:8] = np.asarray(inputs["ab2"], f32).reshape(4, PCH).T
    bpack[:, 8:10] = np.asarray(inputs["eb1"], f32).reshape(2, PCH).T
    bpack[:, 10:12] = np.asarray(inputs["g1"], f32).reshape(2, PCH).T
    bpack[:, 12:14] = np.asarray(inputs["b1"], f32).reshape(2, PCH).T
    bpack[0:D_HID, 14] = np.asarray(inputs["eb2"], f32)
    bpack[0:D_HID, 15] = np.asarray(inputs["g2"], f32)
    bpack[0:D_HID, 16] = np.asarray(inputs["b2"], f32)
    rpack = np.ones((1, PCH + BT), bf16)

    shared = {
        "wpack": np.ascontiguousarray(wpack),
        "bpack": np.ascontiguousarray(bpack),
        "caug": np.ascontiguousarray(bf(caug)),
        "ew2r": np.ascontiguousarray(np.asarray(inputs["ew2"], f32)),
        "rpack": np.ascontiguousarray(rpack),
    }
    in_maps = []
    for c in range(NCORES):
        m = dict(shared)
        m["xT"] = np.ascontiguousarray(xT_full[:, c * BS : (c + 1) * BS])
        in_maps.append(m)
    return in_maps


def _run(inputs, trace=False):
    nc = _get_nc()
    in_maps = _prep_in_maps(inputs)
    res = run_bass_kernel_spmd(
        nc, in_maps, core_ids=list(range(NCORES)), trace=trace
    )
    attn = np.empty((B, D_IN), np.float32)
    enc = np.empty((B, D_HID), np.float32)
    memb = np.empty((B, D_HID), np.float32)
    for c in range(NCORES):
        r = res.results[c]
        sl = slice(c * BS, (c + 1) * BS)
        attn[sl, :] = np.asarray(r["attnT"]).astype(np.float32).T
        enc[sl, :] = np.asarray(r["encT"]).astype(np.float32).T
        memb[sl, :] = np.asarray(r["membT"]).astype(np.float32).T
    return (attn, enc, memb), res


def kernel(**inputs):
    out, _ = _run(inputs, trace=False)
    return out


# revision 27
# speedup vs baseline: 1.1427x; 1.0962x over previous
"""Trainium2 Bass kernel for nn_AFDEKM_Model (attention-gated encoder + fuzzy
c-means membership), data-parallel over 8 NeuronCores.

Layout: feature-major on device — activations live as [features, batch_tile]
so Linear layers need no transposes (weights [in, out] are already the lhsT
the TensorEngine wants), BatchNorm batch-reductions are free-axis DVE ops, and
BN apply is one fused ScalarEngine activation per tile. Softmax over features
(partition axis) uses ones-matmul column sums plus a K=1 broadcast matmul.
The host transposes x once and transposes the three outputs back.

Matmul operands are bf16 (full-rate PE path; measured 216ns/MM warm at N=512),
PSUM accumulation fp32. Tiles are processed in groups with software-pipelined
emission (group g's normalize/encode stage is emitted after group g+1's
matmul stages) so the TensorEngine never idles long enough for the HAM clock
gate to re-throttle it, and the ScalarEngine runs same-LUT activations
back-to-back (table reloads cost ~1.3us). Training-mode BatchNorm over the
full 65536-row batch: per-core partial stats are combined with two tiny
AllReduces.
"""

import sys
import types

import numpy as np

B, D_IN, D_MID, D_HID, K = 65536, 512, 256, 64, 64
BN_EPS = 1e-5
NCORES = 8
BS = B // NCORES          # 8192 rows per core
BT = 512                  # batch tile (free dim; == PSUM bank / BN_STATS_FMAX)
NT = BS // BT             # 16 tiles per core
GRP = 4                   # tiles per ACT-table group
PCH = 128                 # partition chunk

_DIN_CH = D_IN // PCH     # 4
_DMID_CH = D_MID // PCH   # 2


def _shim_antenv_hooks():
    # this container's antenv lacks axon_hooks; bass_utils imports it when
    # trace=True. Harmless no-op module unless test.py installs the real one.
    if "antenv.axon_hooks" not in sys.modules:
        mod = types.ModuleType("antenv.axon_hooks")
        holder = {}
        mod.set_axon_ntff_profile_hook = lambda h: holder.__setitem__("h", h)
        mod.get_axon_ntff_profile_hook = lambda: holder.get("h")
        sys.modules["antenv.axon_hooks"] = mod


_shim_antenv_hooks()

import concourse.mybir as mybir  # noqa: E402
import concourse.bass as bass  # noqa: E402
import concourse.tile as tile  # noqa: E402
from concourse.bass_utils import run_bass_kernel_spmd  # noqa: E402

F32 = mybir.dt.float32
BF16 = mybir.dt.bfloat16
AF = mybir.ActivationFunctionType
ALU = mybir.AluOpType

_MAX_WAITS = 1  # this neuronxcc build allows one sync-wait per instruction
_wait_ctr = [0]


def _split_excess_waits(nc):
    """Spill excess semaphore waits onto same-engine NoOps (walrus here
    rejects instructions with >1 sync wait)."""
    for fn in nc.m.functions:
        for bb in fn.blocks:
            idx = 0
            while idx < len(bb.instructions):
                ins = bb.instructions[idx]
                si = ins.sync_info
                if si is None:
                    idx += 1
                    continue
                waits = list(si.on_wait)
                if len(waits) <= _MAX_WAITS:
                    idx += 1
                    continue
                keep = waits[-_MAX_WAITS:]
                extra = waits[:-_MAX_WAITS]
                n_ins = 0
                for i in range(0, len(extra), _MAX_WAITS):
                    chunk = extra[i : i + _MAX_WAITS]
                    _wait_ctr[0] += 1
                    nop = mybir.InstNoOp(
                        name=f"{ins.name}-wspill{_wait_ctr[0]}",
                        sync_info=mybir.SyncInfo(on_wait=chunk, on_update=[]),
                        bass_nofuse=True,
                        engine=ins.engine,
                    )
                    nc.register_instruction(nop, overwrite=True)
                    bb.instructions.insert(idx + n_ins, nop)
                    n_ins += 1
                ins.sync_info = mybir.SyncInfo(
                    on_wait=keep, on_update=list(si.on_update)
                )
                idx += n_ins + 1


def _recip(nc, out_ap, in_ap):
    """1/x on the ScalarEngine LUT (HW-measured ~1e-5 max rel err over
    1e-2..1e6 — plenty for softmax/membership normalizers). bass's
    activation() refuses Reciprocal wholesale, so emit InstActivation
    directly."""
    eng = nc.scalar
    inputs = [eng.lower_ap(in_ap)]
    for arg in (0.0, 1.0, 0.0):  # bias, scale, alpha
        inputs.append(mybir.ImmediateValue(dtype=mybir.dt.float32, value=float(arg)))
    return eng.add_instruction(
        mybir.InstActivation(
            name=eng.bass.get_next_instruction_name(),
            func=AF.Reciprocal,
            ins=inputs,
            outs=[eng.lower_ap(out_ap)],
        )
    )


def build():
    nc = bass.Bass()

    # ---- I/O ----
    xT = nc.declare_dram_parameter("xT", [D_IN, BS], BF16, isOutput=False)
    # all [128, *] weights packed into one tensor -> one startup DMA
    # cols: aw1 (4x512) | aw2 (4x512) | ew1 (4x256) | ew2 (2x64) | ones_col (1)
    WPACK = 4 * D_IN + 4 * D_IN + 4 * D_MID + 2 * D_HID + 1
    wpack = nc.declare_dram_parameter("wpack", [PCH, WPACK], BF16, isOutput=False)
    # biases/gammas packed: ab1 (4) | ab2 (4) | eb1 (2) | g1 (2) | b1 (2) |
    # eb2 | g2 | b2  (last three on partitions 0:64)
    bpack = nc.declare_dram_parameter("bpack", [PCH, 17], F32, isOutput=False)
    caug = nc.declare_dram_parameter("caug", [D_HID + 2, K], BF16, isOutput=False)
    # ones_row (128) | ones_bt (512) on one partition
    rpack = nc.declare_dram_parameter("rpack", [1, PCH + BT], BF16, isOutput=False)
    # encoder layer-2 weights in float32r: the e2 matmul is tiny (2 per tile)
    # and running it at full fp32 precision halves enc's worst-case error
    F32R = mybir.dt.float32r
    ew2r = nc.declare_dram_parameter("ew2r", [D_MID, D_HID], F32R, isOutput=False)

    attnT = nc.declare_dram_parameter("attnT", [D_IN, BS], BF16, isOutput=True)
    encT = nc.declare_dram_parameter("encT", [D_HID, BS], BF16, isOutput=True)
    membT = nc.declare_dram_parameter("membT", [D_HID, BS], BF16, isOutput=True)

    with tile.TileContext(nc) as tc:
        with (
            tc.tile_pool(name="wp", bufs=1) as wp,
            tc.tile_pool(name="rp", bufs=1) as rp,
            tc.tile_pool(name="tp", bufs=2) as tp,
            tc.tile_pool(name="pmm", bufs=4, space="PSUM") as pmm,
            tc.tile_pool(name="ps1", bufs=4, space="PSUM") as ps1,
            tc.tile_pool(name="prb", bufs=1, space="PSUM") as prb,
            tc.tile_pool(name="dp", bufs=1, space="DRAM") as dp,
        ):
            # ---- load weights / constants (one DMA each) ----
            wpack_sb = wp.tile([PCH, 4 * D_IN + 4 * D_IN + 4 * D_MID + 2 * D_HID + 1], BF16, name="wpack_sb")
            nc.sync.dma_start(out=wpack_sb[:, :], in_=wpack[:, :])
            bpack_sb = wp.tile([PCH, 17], F32, name="bpack_sb")
            nc.sync.dma_start(out=bpack_sb[:, :], in_=bpack[:, :])
            caug_sb = wp.tile([D_HID + 2, K], BF16, name="caug_sb")
            nc.sync.dma_start(out=caug_sb[:, :], in_=caug[:, :])
            rpack_sb = wp.tile([1, PCH + BT], BF16, name="rpack_sb")
            nc.sync.dma_start(out=rpack_sb[:, :], in_=rpack[:, :])
            ew2r_sb = wp.tile([PCH, _DMID_CH, D_HID], F32R, name="ew2r_sb")
            nc.sync.dma_start(
                out=ew2r_sb[:, :, :],
                in_=ew2r[:, :].rearrange("(c p) h -> p c h", c=_DMID_CH),
            )

            o = [0]

            def _wslice(n):
                a = wpack_sb[:, o[0] : o[0] + n]
                o[0] += n
                return a

            aw1_sb = [_wslice(D_IN) for _ in range(_DIN_CH)]
            aw2_sb = [_wslice(D_IN) for _ in range(_DIN_CH)]
            ew1_sb = [_wslice(D_MID) for _ in range(_DIN_CH)]
            ew2_sb = [_wslice(D_HID) for _ in range(_DMID_CH)]
            onec_sb = _wslice(1)
            ab1_sb = [bpack_sb[:, m : m + 1] for m in range(4)]
            ab2_sb = [bpack_sb[:, 4 + m : 5 + m] for m in range(4)]
            eb1_sb = [bpack_sb[:, 8 + m : 9 + m] for m in range(2)]
            g1_sb = [bpack_sb[:, 10 + m : 11 + m] for m in range(2)]
            b1_sb = [bpack_sb[:, 12 + m : 13 + m] for m in range(2)]
            eb2_sb = bpack_sb[0:D_HID, 14:15]
            g2_sb = bpack_sb[0:D_HID, 15:16]
            b2_sb = bpack_sb[0:D_HID, 16:17]
            oner_sb = rpack_sb[:, 0:PCH]
            onesbt_sb = rpack_sb[:, PCH : PCH + BT]
            eps_sb = wp.tile([PCH, 1], F32, name="eps_sb")
            nc.vector.memset(eps_sb[:, :], BN_EPS)

            # ncfw warm-up: first collective pays ~10us trigger latency, so
            # fire a tiny AllReduce early; it overlaps pass A compute.
            cwu_i = dp.tile([1, 2], F32, name="cwu_i")
            cwu_o = dp.tile([1, 2], F32, name="cwu_o")
            nc.gpsimd.dma_start(out=cwu_i[:, :], in_=rpack[:, 0:4].bitcast(F32))
            nc.gpsimd.collective_compute(
                "AllReduce",
                ALU.add,
                replica_groups=[list(range(NCORES))],
                ins=[cwu_i[:, :].opt()],
                outs=[cwu_o[:, :].opt()],
            )

            # ---- residents ----
            t1_res = [
                rp.tile([PCH, BS], BF16, name=f"t1_res_{m}") for m in range(_DMID_CH)
            ]
            t2_res = rp.tile([D_HID, BS], F32, name="t2_res")
            stats1 = [
                rp.tile([PCH, NT * 6], F32, name=f"stats1_{m}")
                for m in range(_DMID_CH)
            ]
            stats2 = rp.tile([D_HID, NT * 6], F32, name="stats2")

            # =========================== PASS A ===========================
            # Stages per tile:
            #   s1: batched x-load, L1 matmuls, relu           (PE + ACT:Relu)
            #   s2: L2 matmuls, exp, column-sum S              (PE + ACT:Exp)
            #   s3: rS = 1/S                                   (ACT:Recip)
            #   s4: broadcast, attn, wf, encoder-L1, BN stats  (PE + DVE)
            # Emission is software-pipelined by group: s1/s2 of group g+1 are
            # emitted before s3/s4 of group g, so the PE always has dense
            # matmul work while ACT switches LUTs.
            state = {}

            def a_s12(t):
                ts = slice(t * BT, (t + 1) * BT)
                xt = tp.tile([PCH, _DIN_CH, BT], BF16, name="xt", tag="xt",
                             bufs=2 * GRP)
                nc.sync.dma_start(
                    out=xt[:, :, :],
                    in_=xT[:, ts].rearrange("(c p) b -> p c b", c=_DIN_CH),
                )
                aT = []
                for m in range(_DIN_CH):
                    ps = pmm.tile([PCH, BT], F32, name="psa", tag="mm")
                    for kc in range(_DIN_CH):
                        nc.tensor.matmul(
                            ps[:, :],
                            aw1_sb[kc][:, m * PCH : (m + 1) * PCH],
                            xt[:, kc, :],
                            start=(kc == 0),
                            stop=(kc == _DIN_CH - 1),
                        )
                    am = tp.tile([PCH, BT], BF16, name="aT", tag=f"aT{m}",
                                 bufs=GRP + 1)
                    nc.scalar.activation(am[:, :], ps[:, :], AF.Relu, bias=ab1_sb[m])
                    aT.append(am)
                ee = []
                ps_s = ps1.tile([1, BT], F32, name="ps_s", tag="s1", bufs=3)
                for m in range(_DIN_CH):
                    ps = pmm.tile([PCH, BT], F32, name="psl", tag="mm")
                    for kc in range(_DIN_CH):
                        nc.tensor.matmul(
                            ps[:, :],
                            aw2_sb[kc][:, m * PCH : (m + 1) * PCH],
                            aT[kc][:, :],
                            start=(kc == 0),
                            stop=(kc == _DIN_CH - 1),
                        )
                    em = tp.tile([PCH, BT], BF16, name="ee", tag=f"ee{m}",
                                 bufs=2 * GRP - 1)
                    # exp without max-subtraction: logits are O(1) here
                    nc.scalar.activation(em[:, :], ps[:, :], AF.Exp, bias=ab2_sb[m])
                    ee.append(em)
                for m in range(_DIN_CH):
                    nc.tensor.matmul(
                        ps_s[:, :],
                        onec_sb[:, :],
                        ee[m][:, :],
                        start=(m == 0),
                        stop=(m == _DIN_CH - 1),
                    )
                state[t] = (xt, ee, ps_s)

            def a_s3(t):
                xt, ee, ps_s = state[t]
                rs = tp.tile([1, BT], BF16, name="rs", tag="rs", bufs=GRP + 1)
                _recip(nc, rs[:, :], ps_s[:, :])
                state[t] = (xt, ee, rs)

            def a_s4(t):
                ts = slice(t * BT, (t + 1) * BT)
                xt, ee, rs = state.pop(t)
                ps_rb = prb.tile([PCH, BT], F32, name="ps_rb", tag="rb")
                nc.tensor.matmul(
                    ps_rb[:, :], oner_sb[:, :], rs[:, :], start=True, stop=True
                )
                rb = tp.tile([PCH, BT], BF16, name="rb", tag="rb", bufs=2)
                nc.vector.tensor_copy(rb[:, :], ps_rb[:, :])
                at4 = tp.tile([PCH, _DIN_CH, BT], BF16, name="attn", tag="attn",
                              bufs=2)
                wf = []
                for m in range(_DIN_CH):
                    nc.vector.tensor_tensor(
                        at4[:, m, :], ee[m][:, :], rb[:, :], ALU.mult
                    )
                    wm = tp.tile([PCH, BT], BF16, name="wf", tag=f"wf{m}")
                    nc.vector.tensor_tensor(
                        wm[:, :], at4[:, m, :], xt[:, m, :], ALU.mult
                    )
                    wf.append(wm)
                nc.sync.dma_start(
                    out=attnT[:, ts].rearrange("(c p) b -> p c b", c=_DIN_CH),
                    in_=at4[:, :, :],
                )
                for m in range(_DMID_CH):
                    ps = pmm.tile([PCH, BT], F32, name="pst1", tag="mm")
                    for kc in range(_DIN_CH):
                        nc.tensor.matmul(
                            ps[:, :],
                            ew1_sb[kc][:, m * PCH : (m + 1) * PCH],
                            wf[kc][:, :],
                            start=(kc == 0),
                            stop=(kc == _DIN_CH - 1),
                        )
                    t1s = t1_res[m][:, ts]
                    nc.vector.tensor_scalar(
                        t1s, ps[:, :], eb1_sb[m][:, :], None, ALU.add
                    )
                    nc.vector.bn_stats(stats1[m][:, t * 6 : (t + 1) * 6], t1s)

            groups = [range(g, min(g + GRP, NT)) for g in range(0, NT, GRP)]
            prev = None
            for grp in groups:
                for t in grp:
                    a_s12(t)
                if prev is not None:
                    for t in prev:
                        a_s3(t)
                    for t in prev:
                        a_s4(t)
                prev = grp
            for t in prev:
                a_s3(t)
            for t in prev:
                a_s4(t)

            # ================== AllReduce BN1 statistics ==================
            # payload per chunk: [mean, E[x^2]] = [mean, var + mean^2]
            pay1 = []
            for m in range(_DMID_CH):
                agg = tp.tile([PCH, 2], F32, name="agg1", tag=f"agg1{m}", bufs=1)
                nc.vector.bn_aggr(agg[:, :], stats1[m][:, :])
                nc.vector.scalar_tensor_tensor(
                    agg[:, 1:2], agg[:, 0:1], agg[:, 0:1], agg[:, 1:2],
                    ALU.mult, ALU.add,
                )
                pay1.append(agg)
            cin1 = dp.tile([PCH, 2 * _DMID_CH], F32, name="cin1")
            cout1 = dp.tile([PCH, 2 * _DMID_CH], F32, name="cout1")
            for m in range(_DMID_CH):
                nc.gpsimd.dma_start(
                    out=cin1[:, 2 * m : 2 * m + 2], in_=pay1[m][:, :]
                )
            nc.gpsimd.collective_compute(
                "AllReduce",
                ALU.add,
                replica_groups=[list(range(NCORES))],
                ins=[cin1[:, :].opt()],
                outs=[cout1[:, :].opt()],
            )

            scale1, shift1 = [], []
            for m in range(_DMID_CH):
                gs = tp.tile([PCH, 2], F32, name="gs1", tag=f"gs1{m}", bufs=1)
                nc.gpsimd.dma_start(out=gs[:, :], in_=cout1[:, 2 * m : 2 * m + 2])
                nc.vector.tensor_scalar(gs[:, :], gs[:, :], 1.0 / NCORES, None, ALU.mult)
                var = tp.tile([PCH, 1], F32, name="var1", tag=f"var1{m}", bufs=1)
                # (mean*mean) - msq = -var ; Sqrt activation flips the sign
                nc.vector.scalar_tensor_tensor(
                    var[:, :], gs[:, 0:1], gs[:, 0:1], gs[:, 1:2],
                    ALU.mult, ALU.subtract,
                )
                std = tp.tile([PCH, 1], F32, name="std1", tag=f"std1{m}", bufs=1)
                nc.scalar.activation(
                    std[:, :], var[:, :], AF.Sqrt, bias=eps_sb[:, :], scale=-1.0
                )
                rstd = tp.tile([PCH, 1], F32, name="rstd1", tag=f"rstd1{m}", bufs=1)
                nc.vector.reciprocal(rstd[:, :], std[:, :])
                sc = tp.tile([PCH, 1], F32, name="sc1", tag=f"sc1{m}", bufs=1)
                nc.vector.tensor_tensor(sc[:, :], g1_sb[m][:, :], rstd[:, :], ALU.mult)
                sh = tp.tile([PCH, 1], F32, name="sh1", tag=f"sh1{m}", bufs=1)
                nc.vector.scalar_tensor_tensor(
                    sh[:, :], gs[:, 0:1], sc[:, :], b1_sb[m][:, :],
                    ALU.mult, ALU.subtract,
                )
                nc.vector.tensor_scalar(sh[:, :], sh[:, :], -1.0, None, ALU.mult)
                scale1.append(sc)
                shift1.append(sh)

            # =========================== PASS B ===========================
            for t in range(NT):
                ts = slice(t * BT, (t + 1) * BT)
                h1 = []
                for m in range(_DMID_CH):
                    hm = tp.tile([PCH, BT], F32R, name="h1", tag=f"h1{m}")
                    nc.scalar.activation(
                        hm[:, :], t1_res[m][:, ts], AF.Relu,
                        bias=shift1[m][:, :], scale=scale1[m][:, :],
                    )
                    h1.append(hm)
                ps = pmm.tile([D_HID, BT], F32, name="pst2", tag="mm")
                for kc in range(_DMID_CH):
                    nc.tensor.matmul(
                        ps[:, :],
                        ew2r_sb[:, kc, :],
                        h1[kc][:, :],
                        start=(kc == 0),
                        stop=(kc == _DMID_CH - 1),
                    )
                t2s = t2_res[:, ts]
                nc.vector.tensor_scalar(t2s, ps[:, :], eb2_sb[:, :], None, ALU.add)
                nc.vector.bn_stats(stats2[:, t * 6 : (t + 1) * 6], t2s)

            # ================== AllReduce BN2 statistics ==================
            agg2 = tp.tile([D_HID, 2], F32, name="agg2", tag="agg2", bufs=1)
            nc.vector.bn_aggr(agg2[:, :], stats2[:, :])
            nc.vector.scalar_tensor_tensor(
                agg2[:, 1:2], agg2[:, 0:1], agg2[:, 0:1], agg2[:, 1:2],
                ALU.mult, ALU.add,
            )
            cin2 = dp.tile([D_HID, 2], F32, name="cin2")
            cout2 = dp.tile([D_HID, 2], F32, name="cout2")
            nc.gpsimd.dma_start(out=cin2[:, :], in_=agg2[:, :])
            nc.gpsimd.collective_compute(
                "AllReduce",
                ALU.add,
                replica_groups=[list(range(NCORES))],
                ins=[cin2[:, :].opt()],
                outs=[cout2[:, :].opt()],
            )
            gs2 = tp.tile([D_HID, 2], F32, name="gs2", tag="gs2", bufs=1)
            nc.gpsimd.dma_start(out=gs2[:, :], in_=cout2[:, :])
            nc.vector.tensor_scalar(gs2[:, :], gs2[:, :], 1.0 / NCORES, None, ALU.mult)
            var2 = tp.tile([D_HID, 1], F32, name="var2", tag="var2", bufs=1)
            nc.vector.scalar_tensor_tensor(
                var2[:, :], gs2[:, 0:1], gs2[:, 0:1], gs2[:, 1:2],
                ALU.mult, ALU.subtract,
            )
            std2 = tp.tile([D_HID, 1], F32, name="std2", tag="std2", bufs=1)
            nc.scalar.activation(
                std2[:, :], var2[:, :], AF.Sqrt, bias=eps_sb[0:D_HID, :], scale=-1.0
            )
            rstd2 = tp.tile([D_HID, 1], F32, name="rstd2", tag="rstd2", bufs=1)
            nc.vector.reciprocal(rstd2[:, :], std2[:, :])
            scale2 = tp.tile([D_HID, 1], F32, name="scale2", tag="scale2", bufs=1)
            nc.vector.tensor_tensor(scale2[:, :], g2_sb[:, :], rstd2[:, :], ALU.mult)
            shift2 = tp.tile([D_HID, 1], F32, name="shift2", tag="shift2", bufs=1)
            nc.vector.scalar_tensor_tensor(
                shift2[:, :], gs2[:, 0:1], scale2[:, :], b2_sb[:, :],
                ALU.mult, ALU.subtract,
            )
            nc.vector.tensor_scalar(shift2[:, :], shift2[:, :], -1.0, None, ALU.mult)

            # =========================== PASS C ===========================
            # rhs_aug rows: [enc (64); enorm (1); ones (1)];
            # caug = [-2*C^T ; ones row ; ||c||^2 row]  so one matmul gives
            # d2 = ||enc||^2 + ||c||^2 - 2 enc.c . Membership = (1/d2)
            # normalized over clusters (m_fuzz=2 -> power 2; the reference's
            # sqrt + 1e-10 clamp cancels; d2 >> 1e-20 for this data).
            cstate = {}

            def c_s1(t):
                ts = slice(t * BT, (t + 1) * BT)
                aug = tp.tile([D_HID + 2, BT], BF16, name="aug", tag="aug",
                              bufs=2 * GRP - 1)
                nc.gpsimd.dma_start(out=aug[D_HID + 1 : D_HID + 2, :], in_=rpack[:, PCH : PCH + BT])
                enc = aug[0:D_HID, :]
                nc.scalar.activation(
                    enc, t2_res[:, ts], AF.Tanh,
                    bias=shift2[:, :], scale=scale2[:, :],
                )
                nc.gpsimd.dma_start(out=encT[:, ts], in_=enc)
                esq = tp.tile([D_HID, BT], BF16, name="esq", tag="esq", bufs=3)
                nc.vector.tensor_tensor(
                    esq[:, :], aug[0:D_HID, :], aug[0:D_HID, :], ALU.mult
                )
                ps_en = ps1.tile([1, BT], F32, name="ps_en", tag="s1", bufs=3)
                nc.tensor.matmul(
                    ps_en[:, :], onec_sb[0:D_HID, :], esq[:, :],
                    start=True, stop=True,
                )
                nc.vector.tensor_copy(aug[D_HID : D_HID + 1, :], ps_en[:, :])
                ps_d2 = pmm.tile([K, BT], F32, name="ps_d2", tag="mm")
                nc.tensor.matmul(
                    ps_d2[:, :], caug_sb[:, :], aug[:, :], start=True, stop=True
                )
                cstate[t] = ps_d2

            def c_s2(t):
                ps_d2 = cstate[t]
                inv = tp.tile([K, BT], BF16, name="inv", tag="inv", bufs=GRP + 1)
                _recip(nc, inv[:, :], ps_d2[:, :])
                ps_s2 = ps1.tile([1, BT], F32, name="ps_s2", tag="s1", bufs=3)
                nc.tensor.matmul(
                    ps_s2[:, :], onec_sb[0:K, :], inv[:, :], start=True, stop=True
                )
                rs2 = tp.tile([1, BT], BF16, name="rs2", tag="rs", bufs=GRP + 1)
                _recip(nc, rs2[:, :], ps_s2[:, :])
                cstate[t] = (inv, rs2)

            def c_s3(t):
                ts = slice(t * BT, (t + 1) * BT)
                inv, rs2 = cstate.pop(t)
                ps_rb2 = prb.tile([K, BT], F32, name="ps_rb2", tag="rb")
                nc.tensor.matmul(
                    ps_rb2[:, :], oner_sb[:, 0:K], rs2[:, :], start=True, stop=True
                )
                mb = tp.tile([K, BT], BF16, name="mb", tag="mb", bufs=3)
                nc.vector.tensor_tensor(
                    mb[:, :], inv[:, :], ps_rb2[:, :], ALU.mult
                )
                nc.gpsimd.dma_start(out=membT[:, ts], in_=mb[:, :])

            prev = None
            for grp in groups:
                for t in grp:
                    c_s1(t)
                if prev is not None:
                    for t in prev:
                        c_s2(t)
                    for t in prev:
                        c_s3(t)
                prev = grp
            for t in prev:
                c_s2(t)
            for t in prev:
                c_s3(t)

    _split_excess_waits(nc)
    return nc


_built = None


def _get_nc():
    global _built
    if _built is None:
        _built = build()
    return _built


def _prep_in_maps(inputs):
    import ml_dtypes

    f32 = np.float32
    bf16 = ml_dtypes.bfloat16
    x = np.asarray(inputs["x"], f32)
    xT_full = np.ascontiguousarray(x.T.astype(bf16))  # [512, 65536]

    centers = np.asarray(inputs["centers"], f32)
    cnorm = (centers.astype(np.float64) ** 2).sum(axis=1).astype(f32)
    caug = np.concatenate(
        [-2.0 * centers.T, np.ones((1, K), f32), cnorm[None, :]], axis=0
    )

    def bf(v):
        return np.asarray(v, f32).astype(bf16)

    wpack = np.concatenate(
        [bf(inputs["aw1"]).reshape(4, PCH, D_IN).transpose(1, 0, 2).reshape(PCH, -1),
         bf(inputs["aw2"]).reshape(4, PCH, D_IN).transpose(1, 0, 2).reshape(PCH, -1),
         bf(inputs["ew1"]).reshape(4, PCH, D_MID).transpose(1, 0, 2).reshape(PCH, -1),
         bf(inputs["ew2"]).reshape(2, PCH, D_HID).transpose(1, 0, 2).reshape(PCH, -1),
         np.ones((PCH, 1), bf16)],
        axis=1,
    )
    bpack = np.zeros((PCH, 17), f32)
    bpack[:, 0:4] = np.asarray(inputs["ab1"], f32).reshape(4, PCH).T
    bpack[:, 4:8] = np.asarray(inputs["ab2"], f32).reshape(4, PCH).T
    bpack[:, 8:10] = np.asarray(inputs["eb1"], f32).reshape(2, PCH).T
    bpack[:, 10:12] = np.asarray(inputs["g1"], f32).reshape(2, PCH).T
    bpack[:, 12:14] = np.asarray(inputs["b1"], f32).reshape(2, PCH).T
    bpack[0:D_HID, 14] = np.asarray(inputs["eb2"], f32)
    bpack[0:D_HID, 15] = np.asarray(inputs["g2"], f32)
    bpack[0:D_HID, 16] = np.asarray(inputs["b2"], f32)
    rpack = np.ones((1, PCH + BT), bf16)

    shared = {
        "wpack": np.ascontiguousarray(wpack),
        "bpack": np.ascontiguousarray(bpack),
        "caug": np.ascontiguousarray(bf(caug)),
        "ew2r": np.ascontiguousarray(np.asarray(inputs["ew2"], f32)),
        "rpack": np.ascontiguousarray(rpack),
    }
    in_maps = []
    for c in range(NCORES):
        m = dict(shared)
        m["xT"] = np.ascontiguousarray(xT_full[:, c * BS : (c + 1) * BS])
        in_maps.append(m)
    return in_maps


def _run(inputs, trace=False):
    nc = _get_nc()
    in_maps = _prep_in_maps(inputs)
    res = run_bass_kernel_spmd(
        nc, in_maps, core_ids=list(range(NCORES)), trace=trace
    )
    attn = np.empty((B, D_IN), np.float32)
    enc = np.empty((B, D_HID), np.float32)
    memb = np.empty((B, D_HID), np.float32)
    for c in range(NCORES):
        r = res.results[c]
        sl = slice(c * BS, (c + 1) * BS)
        attn[sl, :] = np.asarray(r["attnT"]).astype(np.float32).T
        enc[sl, :] = np.asarray(r["encT"]).astype(np.float32).T
        memb[sl, :] = np.asarray(r["membT"]).astype(np.float32).T
    return (attn, enc, memb), res


def kernel(**inputs):
    out, _ = _run(inputs, trace=False)
    return out
